# revision 20
# baseline (speedup 1.0000x reference)
"""Trainium2 Bass kernel for nn_ConnectLoss (pairwise BCE+Dice loss with greedy assignment).

Strategy: shard the flattened pixel axis M = B*H*W across the 8 NeuronCores.
Each core reduces its M/8 pixel shard to a [3, 17, 17] tensor of segment sums
via a one-hot GEMM on the tensor engine:

    S[q, n, k] = sum_m 1[t==n] * X_q[k, m]   for X_q in {p, log p, log(1-p)}

All totals (sum_p, total log(1-p)) come free as sums over the 17 classes,
which partition the pixels, so no ones row/column is needed.  cnt[n] is a
host-side bincount of the integer target.

Engine split per tile (fp16 end to end, chosen to balance the engines):
  - DMA loads host-prestaged fp16 p directly into the moving tile's p slots
    (no on-device copy / transposing DMA).
  - DVE computes log p with a one-pass bit trick in 4x perf mode:
    ln(p) ~= bits(p_fp16) * (ln2/1024) - ln2*(15 - sigma), where sigma centers
    the mantissa-interpolation error at zero mean for uniform p.  The residual
    (+-0.03, zero mean) only enters bce via segment sums / M -- harmless.
  - DVE builds the 17-plane one-hot with a single broadcast is_equal.
  - The scalar (Act) engine computes only ONE exact Ln pass: log(1+eps-p).
  - PE streams one 306-column matmul per 128-pixel group (block-diagonal
    GROUP=6 packing), accumulating the whole shard into one PSUM bank.

Host staging clamps p to [6.2e-5, 1-2^-11] (fp16-normal range; the reference
clips at [1e-7, 1-1e-7] -- the difference is far below the loss tolerance)
and lays tensors out as the exact SBUF image so every DMA is wide and clean.
"""

import sys

_REPO = "/root/.axon_site/_ro/trn_rl_repo"
if _REPO not in sys.path:
    sys.path.insert(0, _REPO)

import numpy as np

EPS = 1e-7
N_INST = 16
B, K, H, W = 4, 17, 768, 768
M = B * H * W  # 2359296
N_CORES = 8
MS = M // N_CORES  # 294912 pixels per core
PART = 128
CPP = MS // PART  # 2304 pixel groups of 128
GROUP = 6  # chunks per matmul (block-diagonal packing)
NB = CPP // GROUP  # 384 blocks of GROUP chunks per core
# ramp-up/-down tile sizes (in blocks): a small first tile gets the tensor
# engine streaming ~8us earlier by not queueing behind 4 full-tile DMAs
TILE_NGS = [8, 16, 32, 48, 48, 48, 48, 48, 48, 24, 16]
assert sum(TILE_NGS) == NB
MERGED_MM = False  # one 306-col matmul per group (multi-dim moving AP)

# log2(1+x) ~= x + SIGMA for x ~ U[0,1): zero-mean interpolation error
SIGMA = 2.0 - 1.0 / np.log(2.0) - 0.5
BIT_SCALE = float(np.log(2.0) / 1024.0)
BIT_BIAS = float(np.log(2.0) * (15.0 - SIGMA))
P_LO = 6.2e-5  # > fp16 min normal, keeps the bit trick exact
P_HI = 1.0 - 2.0**-11

_CACHE = {}


def _build_program():
    import concourse.tile as tile
    from concourse import bacc, mybir

    f32 = mybir.dt.float32
    f16 = mybir.dt.float16
    u16 = mybir.dt.uint16
    Alu = mybir.AluOpType
    Act = mybir.ActivationFunctionType

    nc = bacc.Bacc("TRN2", target_bir_lowering=False, debug=False, num_devices=N_CORES)

    pred_ap = nc.dram_tensor(
        "pred", [PART, NB, K, GROUP], f16, kind="ExternalInput"
    ).ap()
    tgt_ap = nc.dram_tensor(
        "tgt", [PART, NB, 1, GROUP], f16, kind="ExternalInput"
    ).ap()
    out_ap = nc.dram_tensor(
        "out", [K * GROUP, 3 * K * GROUP], f32, kind="ExternalOutput"
    ).ap()

    with tile.TileContext(nc) as tc:
        with (
            tc.tile_pool(name="io", bufs=4) as io_pool,
            tc.tile_pool(name="const", bufs=1) as const_pool,
            tc.tile_pool(name="acc", bufs=1, space="PSUM") as psum_pool,
            tc.tile_pool(name="res", bufs=1) as res_pool,
        ):
            # J[p, 0, j, s] = j -- broadcast comparand for the one-hot
            J = const_pool.tile([PART, 1, K, GROUP], f16)
            for j in range(K):
                nc.gpsimd.memset(J[:, 0, j, :], float(j))
            # activation() resolves float biases through the const-AP
            # database; the Ln bias isn't among the defaults.  Registering a
            # pool tile keeps the write -> first-activation ordering inside
            # the tile dependency tracker (no all-engine barrier gating DMA).
            ln_bias = const_pool.tile([PART, 1], f32)
            nc.gpsimd.memset(ln_bias[:], 1.0 + EPS)
            nc.const_aps.aps[(f32, 1.0 + EPS)] = ln_bias[:]

            # Per tile: dense P (one contiguous DMA descriptor per partition),
            # X2 holds [log p (DVE bit trick), log(1+eps-p) (Act Ln)].  Two
            # matmuls per group share the stationary one-hot T_g and stream
            # 102 + 204 columns into disjoint PSUM column ranges.
            out_sb = res_pool.tile([K * GROUP, 3 * K * GROUP], f32)
            if MERGED_MM:
                S_psum = psum_pool.tile([K * GROUP, 3 * K * GROUP], f32)
                off = 0
                for ti, ng in enumerate(TILE_NGS):
                    sl = slice(off, off + ng)
                    last = ti == len(TILE_NGS) - 1
                    # plane-set-major X: [q, g, k, s]; p lands contiguous
                    X = io_pool.tile([PART, 3, ng, K, GROUP], f16, name="X")
                    tt = io_pool.tile([PART, ng, 1, GROUP], f16, name="tt")
                    nc.sync.dma_start(tt[:], tgt_ap[:, sl])
                    nc.sync.dma_start(X[:, 0], pred_ap[:, sl])

                    T = io_pool.tile([PART, ng, K, GROUP], f16, name="T")
                    nh = ng // 2
                    for h in range(2):
                        hs = slice(h * nh, (h + 1) * nh)
                        nc.vector.tensor_tensor(
                            T[:, hs],
                            tt[:, hs].broadcast_to([PART, nh, K, GROUP]),
                            J[:].broadcast_to([PART, nh, K, GROUP]),
                            Alu.is_equal,
                        )
                    for h in range(2):
                        hs = slice(h * nh, (h + 1) * nh)
                        nc.vector.tensor_scalar(
                            X[:, 1, hs],
                            X[:, 0, hs].bitcast(u16),
                            BIT_SCALE,
                            BIT_BIAS,
                            Alu.mult,
                            Alu.subtract,
                        )
                        nc.scalar.activation(
                            X[:, 2, hs], X[:, 0, hs], Act.Ln,
                            bias=1.0 + EPS, scale=-1.0,
                        )

                    for g in range(ng):
                        nc.tensor.matmul(
                            S_psum[:],
                            T[:, g],
                            X[:, :, g],
                            start=(ti == 0 and g == 0),
                            stop=(last and g == ng - 1),
                        )
                    off += ng

                nc.scalar.copy(out_sb[:], S_psum[:])
                nc.sync.dma_start(out_ap[:], out_sb[:])
            else:
                S_psumA = psum_pool.tile([K * GROUP, K * GROUP], f32)
                S_psumB = psum_pool.tile([K * GROUP, 2 * K * GROUP], f32)
                off = 0
                for ti, ng in enumerate(TILE_NGS):
                    sl = slice(off, off + ng)
                    last = ti == len(TILE_NGS) - 1
                    P = io_pool.tile([PART, ng, K, GROUP], f16, name="P")
                    tt = io_pool.tile([PART, ng, 1, GROUP], f16, name="tt")
                    # tt rides the gpsimd DGE ring so P owns the SP queue
                    nc.gpsimd.dma_start(tt[:], tgt_ap[:, sl])
                    nc.sync.dma_start(P[:], pred_ap[:, sl])

                    T = io_pool.tile([PART, ng, K, GROUP], f16, name="T")
                    X2 = io_pool.tile([PART, ng, 2, K, GROUP], f16, name="X2")
                    nh = ng // 2
                    for h in range(2):
                        hs = slice(h * nh, (h + 1) * nh)
                        nc.vector.tensor_tensor(
                            T[:, hs],
                            tt[:, hs].broadcast_to([PART, nh, K, GROUP]),
                            J[:].broadcast_to([PART, nh, K, GROUP]),
                            Alu.is_equal,
                        )
                    for h in range(2):
                        hs = slice(h * nh, (h + 1) * nh)
                        nc.vector.tensor_scalar(
                            X2[:, hs, 0, :, :],
                            P[:, hs].bitcast(u16),
                            BIT_SCALE,
                            BIT_BIAS,
                            Alu.mult,
                            Alu.subtract,
                        )
                        nc.scalar.activation(
                            X2[:, hs, 1, :, :], P[:, hs], Act.Ln,
                            bias=1.0 + EPS, scale=-1.0,
                        )

                    # A-matmuls (need only P+T) run while Act/DVE fill X2
                    for g in range(ng):
                        nc.tensor.matmul(
                            S_psumA[:],
                            T[:, g],
                            P[:, g],
                            start=(ti == 0 and g == 0),
                            stop=(last and g == ng - 1),
                        )
                    for g in range(ng):
                        nc.tensor.matmul(
                            S_psumB[:],
                            T[:, g],
                            X2[:, g],
                            start=(ti == 0 and g == 0),
                            stop=(last and g == ng - 1),
                        )
                    off += ng

                # A-chain finishes before the last B-matmuls: drain it early
                nc.vector.tensor_copy(out_sb[:, 0 : K * GROUP], S_psumA[:])
                nc.sync.dma_start(
                    out_ap[:, 0 : K * GROUP], out_sb[:, 0 : K * GROUP]
                )
                nc.vector.tensor_copy(out_sb[:, K * GROUP :], S_psumB[:])
                nc.sync.dma_start(out_ap[:, K * GROUP :], out_sb[:, K * GROUP :])

    nc.compile()
    return nc


def _get_program():
    if "nc" not in _CACHE:
        _CACHE["nc"] = _build_program()
    return _CACHE["nc"]


def _shard_inputs(pred_instance_mask, target_mask):
    pred = np.asarray(pred_instance_mask)
    tgt = np.asarray(target_mask).reshape(M)
    _CACHE["cnt"] = np.bincount(tgt, minlength=K).astype(np.float64)
    p16 = np.clip(pred, P_LO, P_HI).astype(np.float16)
    t16 = tgt.astype(np.float16)
    in_maps = []
    hh = H // 2  # each core owns half of one batch image's rows
    for c in range(N_CORES):
        b, half = divmod(c, 2)
        ps = p16[b, :, half * hh : (half + 1) * hh, :].reshape(K, MS)
        # pixel m = part*CPP + blk*GROUP + s
        px = ps.reshape(K, PART, NB, GROUP).transpose(1, 2, 0, 3)
        ts = t16[c * MS : (c + 1) * MS].reshape(PART, NB, 1, GROUP)
        in_maps.append(
            {"pred": np.ascontiguousarray(px), "tgt": np.ascontiguousarray(ts)}
        )
    return in_maps


def _finish(S):
    """Combine the summed [3, 17, 17] segment-sum tensor into the scalar loss."""
    S3, cnt = S
    tp, S_lp, S_l1 = S3[0], S3[1], S3[2]
    sum_p = tp.sum(axis=0)  # classes partition the pixels
    total_l1 = S_l1.sum(axis=0)
    bce = -(S_lp - S_l1 + total_l1[None, :]) / M
    dice = 1.0 - (2.0 * tp + EPS) / (cnt[:, None] + sum_p[None, :] + EPS)
    L_full = bce + dice  # [target id 0..16, channel 0..16]
    bg = L_full[0, 0]
    L = L_full[1:, 1:]
    avail = np.ones(16, bool)
    total = 0.0
    for n in range(16):
        row = np.where(avail, L[n], np.inf)
        kk = int(np.argmin(row))
        avail[kk] = False
        total += row[kk]
    return (bg + total) / N_INST


def _run(in_maps, trace=False):
    from concourse.bass_utils import run_bass_kernel_spmd

    nc = _get_program()
    res = run_bass_kernel_spmd(nc, in_maps, list(range(N_CORES)), trace=trace)
    S3 = np.zeros((3, K, K), np.float64)
    for c in range(N_CORES):
        # rows j*GROUP+s, cols q*(K*GROUP)+x*GROUP+s'; slot-diagonal only
        full = res.results[c]["out"].astype(np.float64)
        S3 += np.einsum("jsqxs->qjx", full.reshape(K, GROUP, 3, K, GROUP))
    return (S3, _CACHE["cnt"]), res


def kernel(pred_instance_mask, target_mask):
    in_maps = _shard_inputs(pred_instance_mask, target_mask)
    S, _ = _run(in_maps)
    return np.float32(_finish(S))


# revision 22
# speedup vs baseline: 1.0326x; 1.0326x over previous
"""Trainium2 Bass kernel for nn_ConnectLoss (pairwise BCE+Dice loss with greedy assignment).

Strategy: shard the flattened pixel axis M = B*H*W across the 8 NeuronCores.
Each core reduces its M/8 pixel shard to a [3, 17, 17] tensor of segment sums
via a one-hot GEMM on the tensor engine:

    S[q, n, k] = sum_m 1[t==n] * X_q[k, m]   for X_q in {p, log p, log(1-p)}

All totals (sum_p, total log(1-p)) come free as sums over the 17 classes,
which partition the pixels, so no ones row/column is needed.  cnt[n] is a
host-side bincount of the integer target.

Engine split per tile (fp16 end to end, chosen to balance the engines):
  - DMA loads host-prestaged fp16 p directly into the moving tile's p slots
    (no on-device copy / transposing DMA).
  - DVE computes log p with a one-pass bit trick in 4x perf mode:
    ln(p) ~= bits(p_fp16) * (ln2/1024) - ln2*(15 - sigma), where sigma centers
    the mantissa-interpolation error at zero mean for uniform p.  The residual
    (+-0.03, zero mean) only enters bce via segment sums / M -- harmless.
  - DVE builds the 17-plane one-hot with a single broadcast is_equal.
  - The scalar (Act) engine computes only ONE exact Ln pass: log(1+eps-p).
  - PE streams one 306-column matmul per 128-pixel group (block-diagonal
    GROUP=6 packing), accumulating the whole shard into one PSUM bank.

Host staging clamps p to [6.2e-5, 1-2^-11] (fp16-normal range; the reference
clips at [1e-7, 1-1e-7] -- the difference is far below the loss tolerance)
and lays tensors out as the exact SBUF image so every DMA is wide and clean.
"""

import sys

_REPO = "/root/.axon_site/_ro/trn_rl_repo"
if _REPO not in sys.path:
    sys.path.insert(0, _REPO)

import numpy as np

EPS = 1e-7
N_INST = 16
B, K, H, W = 4, 17, 768, 768
M = B * H * W  # 2359296
N_CORES = 8
MS = M // N_CORES  # 294912 pixels per core
PART = 128
CPP = MS // PART  # 2304 pixel groups of 128
GROUP = 6  # chunks per matmul (block-diagonal packing)
NB = CPP // GROUP  # 384 blocks of GROUP chunks per core
# ramp-up/-down tile sizes (in blocks): a small first tile gets the tensor
# engine streaming ~8us earlier by not queueing behind 4 full-tile DMAs
TILE_NGS = [8, 16, 32, 48, 48, 48, 48, 48, 48, 24, 16]
assert sum(TILE_NGS) == NB
MERGED_MM = False  # one 306-col matmul per group (multi-dim moving AP)

# log2(1+x) ~= x + SIGMA for x ~ U[0,1): zero-mean interpolation error
SIGMA = 2.0 - 1.0 / np.log(2.0) - 0.5
BIT_SCALE = float(np.log(2.0) / 1024.0)
BIT_BIAS = float(np.log(2.0) * (15.0 - SIGMA))
P_LO = 6.2e-5  # > fp16 min normal, keeps the bit trick exact
P_HI = 1.0 - 2.0**-11

_CACHE = {}


def _build_program():
    import concourse.tile as tile
    from concourse import bacc, mybir

    f32 = mybir.dt.float32
    f16 = mybir.dt.float16
    u16 = mybir.dt.uint16
    Alu = mybir.AluOpType
    Act = mybir.ActivationFunctionType

    nc = bacc.Bacc("TRN2", target_bir_lowering=False, debug=False, num_devices=N_CORES)

    pred_ap = nc.dram_tensor(
        "pred", [PART, NB, K, GROUP], f16, kind="ExternalInput"
    ).ap()
    tgt_ap = nc.dram_tensor(
        "tgt", [PART, NB, 1, GROUP], f16, kind="ExternalInput"
    ).ap()
    out_ap = nc.dram_tensor(
        "out", [K * GROUP, 3 * K * GROUP], f32, kind="ExternalOutput"
    ).ap()

    with tile.TileContext(nc) as tc:
        with (
            tc.tile_pool(name="io", bufs=4) as io_pool,
            tc.tile_pool(name="const", bufs=1) as const_pool,
            tc.tile_pool(name="acc", bufs=1, space="PSUM") as psum_pool,
            tc.tile_pool(name="res", bufs=1) as res_pool,
        ):
            # J[p, 0, j, s] = j -- broadcast comparand for the one-hot
            J = const_pool.tile([PART, 1, K, GROUP], f16)
            for j in range(K):
                nc.gpsimd.memset(J[:, 0, j, :], float(j))
            # activation() resolves float biases through the const-AP
            # database; the Ln bias isn't among the defaults.  Registering a
            # pool tile keeps the write -> first-activation ordering inside
            # the tile dependency tracker (no all-engine barrier gating DMA).
            ln_bias = const_pool.tile([PART, 1], f32)
            nc.gpsimd.memset(ln_bias[:], 1.0 + EPS)
            nc.const_aps.aps[(f32, 1.0 + EPS)] = ln_bias[:]

            # Per tile: dense P (one contiguous DMA descriptor per partition),
            # X2 holds [log p (DVE bit trick), log(1+eps-p) (Act Ln)].  Two
            # matmuls per group share the stationary one-hot T_g and stream
            # 102 + 204 columns into disjoint PSUM column ranges.
            out_sb = res_pool.tile([K * GROUP, 3 * K * GROUP], f32)
            if MERGED_MM:
                S_psum = psum_pool.tile([K * GROUP, 3 * K * GROUP], f32)
                off = 0
                for ti, ng in enumerate(TILE_NGS):
                    sl = slice(off, off + ng)
                    last = ti == len(TILE_NGS) - 1
                    # plane-set-major X: [q, g, k, s]; p lands contiguous
                    X = io_pool.tile([PART, 3, ng, K, GROUP], f16, name="X")
                    tt = io_pool.tile([PART, ng, 1, GROUP], f16, name="tt")
                    nc.sync.dma_start(tt[:], tgt_ap[:, sl])
                    nc.sync.dma_start(X[:, 0], pred_ap[:, sl])

                    T = io_pool.tile([PART, ng, K, GROUP], f16, name="T")
                    nh = ng // 2
                    for h in range(2):
                        hs = slice(h * nh, (h + 1) * nh)
                        nc.vector.tensor_tensor(
                            T[:, hs],
                            tt[:, hs].broadcast_to([PART, nh, K, GROUP]),
                            J[:].broadcast_to([PART, nh, K, GROUP]),
                            Alu.is_equal,
                        )
                    for h in range(2):
                        hs = slice(h * nh, (h + 1) * nh)
                        nc.vector.tensor_scalar(
                            X[:, 1, hs],
                            X[:, 0, hs].bitcast(u16),
                            BIT_SCALE,
                            BIT_BIAS,
                            Alu.mult,
                            Alu.subtract,
                        )
                        nc.scalar.activation(
                            X[:, 2, hs], X[:, 0, hs], Act.Ln,
                            bias=1.0 + EPS, scale=-1.0,
                        )

                    for g in range(ng):
                        nc.tensor.matmul(
                            S_psum[:],
                            T[:, g],
                            X[:, :, g],
                            start=(ti == 0 and g == 0),
                            stop=(last and g == ng - 1),
                        )
                    off += ng

                nc.scalar.copy(out_sb[:], S_psum[:])
                nc.sync.dma_start(out_ap[:], out_sb[:])
            else:
                S_psumA = psum_pool.tile([K * GROUP, K * GROUP], f32)
                S_psumB = psum_pool.tile([K * GROUP, 2 * K * GROUP], f32)
                off = 0
                for ti, ng in enumerate(TILE_NGS):
                    sl = slice(off, off + ng)
                    last = ti == len(TILE_NGS) - 1
                    P = io_pool.tile([PART, ng, K, GROUP], f16, name="P")
                    tt = io_pool.tile([PART, ng, 1, GROUP], f16, name="tt")
                    nc.sync.dma_start(tt[:], tgt_ap[:, sl])
                    nc.sync.dma_start(P[:], pred_ap[:, sl])

                    T = io_pool.tile([PART, ng, K, GROUP], f16, name="T")
                    X2 = io_pool.tile([PART, ng, 2, K, GROUP], f16, name="X2")
                    nh = ng // 2
                    for h in range(2):
                        hs = slice(h * nh, (h + 1) * nh)
                        nc.vector.tensor_tensor(
                            T[:, hs],
                            tt[:, hs].broadcast_to([PART, nh, K, GROUP]),
                            J[:].broadcast_to([PART, nh, K, GROUP]),
                            Alu.is_equal,
                        )
                    for h in range(2):
                        hs = slice(h * nh, (h + 1) * nh)
                        nc.vector.tensor_scalar(
                            X2[:, hs, 0, :, :],
                            P[:, hs].bitcast(u16),
                            BIT_SCALE,
                            BIT_BIAS,
                            Alu.mult,
                            Alu.subtract,
                        )
                        nc.scalar.activation(
                            X2[:, hs, 1, :, :], P[:, hs], Act.Ln,
                            bias=1.0 + EPS, scale=-1.0,
                        )

                    # A-matmuls (need only P+T) run while Act/DVE fill X2
                    for g in range(ng):
                        nc.tensor.matmul(
                            S_psumA[:],
                            T[:, g],
                            P[:, g],
                            start=(ti == 0 and g == 0),
                            stop=(last and g == ng - 1),
                        )
                    for g in range(ng):
                        nc.tensor.matmul(
                            S_psumB[:],
                            T[:, g],
                            X2[:, g],
                            start=(ti == 0 and g == 0),
                            stop=(last and g == ng - 1),
                        )
                    off += ng

                # A-chain finishes before the last B-matmuls: drain it early
                nc.scalar.copy(out_sb[:, 0 : K * GROUP], S_psumA[:])
                nc.sync.dma_start(
                    out_ap[:, 0 : K * GROUP], out_sb[:, 0 : K * GROUP]
                )
                nc.scalar.copy(out_sb[:, K * GROUP :], S_psumB[:])
                nc.sync.dma_start(out_ap[:, K * GROUP :], out_sb[:, K * GROUP :])

    nc.compile()
    return nc


def _get_program():
    if "nc" not in _CACHE:
        _CACHE["nc"] = _build_program()
    return _CACHE["nc"]


def _shard_inputs(pred_instance_mask, target_mask):
    pred = np.asarray(pred_instance_mask)
    tgt = np.asarray(target_mask).reshape(M)
    _CACHE["cnt"] = np.bincount(tgt, minlength=K).astype(np.float64)
    p16 = np.clip(pred, P_LO, P_HI).astype(np.float16)
    t16 = tgt.astype(np.float16)
    in_maps = []
    hh = H // 2  # each core owns half of one batch image's rows
    for c in range(N_CORES):
        b, half = divmod(c, 2)
        ps = p16[b, :, half * hh : (half + 1) * hh, :].reshape(K, MS)
        # pixel m = part*CPP + blk*GROUP + s
        px = ps.reshape(K, PART, NB, GROUP).transpose(1, 2, 0, 3)
        ts = t16[c * MS : (c + 1) * MS].reshape(PART, NB, 1, GROUP)
        in_maps.append(
            {"pred": np.ascontiguousarray(px), "tgt": np.ascontiguousarray(ts)}
        )
    return in_maps


def _finish(S):
    """Combine the summed [3, 17, 17] segment-sum tensor into the scalar loss."""
    S3, cnt = S
    tp, S_lp, S_l1 = S3[0], S3[1], S3[2]
    sum_p = tp.sum(axis=0)  # classes partition the pixels
    total_l1 = S_l1.sum(axis=0)
    bce = -(S_lp - S_l1 + total_l1[None, :]) / M
    dice = 1.0 - (2.0 * tp + EPS) / (cnt[:, None] + sum_p[None, :] + EPS)
    L_full = bce + dice  # [target id 0..16, channel 0..16]
    bg = L_full[0, 0]
    L = L_full[1:, 1:]
    avail = np.ones(16, bool)
    total = 0.0
    for n in range(16):
        row = np.where(avail, L[n], np.inf)
        kk = int(np.argmin(row))
        avail[kk] = False
        total += row[kk]
    return (bg + total) / N_INST


def _run(in_maps, trace=False):
    from concourse.bass_utils import run_bass_kernel_spmd

    nc = _get_program()
    res = run_bass_kernel_spmd(nc, in_maps, list(range(N_CORES)), trace=trace)
    S3 = np.zeros((3, K, K), np.float64)
    for c in range(N_CORES):
        # rows j*GROUP+s, cols q*(K*GROUP)+x*GROUP+s'; slot-diagonal only
        full = res.results[c]["out"].astype(np.float64)
        S3 += np.einsum("jsqxs->qjx", full.reshape(K, GROUP, 3, K, GROUP))
    return (S3, _CACHE["cnt"]), res


def kernel(pred_instance_mask, target_mask):
    in_maps = _shard_inputs(pred_instance_mask, target_mask)
    S, _ = _run(in_maps)
    return np.float32(_finish(S))
